# revision 64
# baseline (speedup 1.0000x reference)
"""Causal self-attention (with the reference's inverted mask) on 8 TRN2
NeuronCores.

Problem (hardcoded): B=2, S=2048, D=1024, H=16 heads, head_dim=64, fp32.
  q/k/v = x @ W* + b*;  score = q k^T / 8;  score += tril(ones)*(-1e9)
  (inverted causal mask: the LOWER triangle incl. diagonal is masked, so
  softmax attends strictly to k > q; row q=S-1 is fully masked and its
  softmax is exactly uniform);  out = softmax(score) @ v @ Wo + bo.

Sharding: core c handles batch b = c//4 and heads [4*(c%4), 4*(c%4)+4).
Each core computes a partial output (its 4 heads' slice of attn @ Wo);
the host sums 4 partials per batch and adds bo.

v3 kernel (all matmuls bf16; inputs pre-swizzled host-side).  The
device is power-limited: HAM duty-cycles the PE to 4/8 under sustained
load, so the design minimizes total PE streaming work and keeps every
engine's queue busy rather than chasing per-matmul peak rates (fp8
DoubleRow measured SLOWER end-to-end: 2 multiplies/cell doubles the
switching power and triples the throttle time).

  Inputs: x arrives as 4 contiguous 1MB DMAs (one per 512-column
    n-slice, n=3 first so phase B's first q-chunk can start early);
    each weight is one contiguous DMA in SBUF tile layout.  All input
    DMAs ride the sync queue in consumption order; small tiles ride
    gpsimd.  A short burst of dummy matmuls covers the first DMAs'
    latency and warms the PE p-state ramp.
  Phase A: QT/KT = W^T x^T in [dh, s] layout (head pairs packed to 128
    partitions), V in [s, dh] layout with an extra ones column per head
    ([V | 1]) so one matmul later yields both the attn numerator and
    the softmax denominator.  Each 512-col QK group is interleaved
    matmul-by-matmul with a 256-col V group so neither side's
    LDWEIGHTS is exposed.
  Phase B (per q-chunk of 512): scores computed TRANSPOSED,
    s^T[k, q] = K^T Q per (head, k-block j).  The two heads of a pair
    contract only K=64 partitions each, so they run as row-tiled
    CONCURRENT matmuls writing the two banks of one [128, 2, 512] PSUM
    tile -- score streaming cost is halved vs a zero-padded K=128
    contraction, and one strided exp per (j, pair) covers both heads.
    p^T = exp(s^T/8) in bf16: off-diagonal tiles split between scalar
    ACT (exact) and DVE Schraudolph (int16 bit trick, ~3.3 percent);
    diagonal tiles exp on scalar and get an affine_select on gpsimd
    (zero where k <= q, no mask tile needed), narrowed to the 128(d+1)
    live columns.  attn^T[dh|sum, q] accumulates matmul([V|1], p^T)
    over j in PSUM per head; attn emission lags the scores by 2 j-items
    (software pipeline over the strict-FIFO PE queue).
    The globally-masked last row (q=2047) is recomputed exactly on the
    host; N=2 [0|1]-column matmuls keep its on-chip denominator finite.
    Normalization per head: sums row -> bf16 SBUF (scalar), K=1 ones
    matmul broadcasts it to 64 partitions, DVE fast reciprocal +
    multiply; odd heads bounce through SBUF + a partition-shifting
    gpsimd DMA into rows 64:128 of the pair tile so phase C contracts
    K=128.  Odd heads run first so the ~2us bounce latency overlaps
    the even heads' chains; the last chunk's norm is split into two
    q-halves so phase C can start while the second half normalizes.
  Phase C (fused per q-chunk): out_partial = attn^T.T @ Wo-rows, heads
    packed in pairs so the contraction runs K=128.  Both 512-col
    output halves of an s-block fill one 2-bank PSUM tile, evacuated
    in parallel on scalar+DVE, written back on sync+gpsimd queues.
    Cross-chunk: 4 score items of the next chunk are emitted around
    the current chunk's norm/C to keep the PE busy through the
    boundary.  PSUM budget: 4 banks attn accumulators + one 2-slot
    pool (4 banks) shared by score tiles, sum-broadcast tiles and
    phase C outputs.
"""

import numpy as np

B, S, D, H, DH = 2, 2048, 1024, 16, 64
HPC = 4                 # heads per core
NCORES = 8
NPAIR = HPC // 2        # head pairs per core (2)
SBLK = S // 128         # 16 s/k blocks
NCH = S // 512          # 4 q-chunks of 512
CHUNKS = D // 128       # 8 contraction chunks of the model dim
WARMUP = 40             # HAM-warming dummy matmuls at kernel start
# Schraudolph fast-exp constants: exp(s/8) ~= bf16(bits = A*s + B) with
# bits computed as int16 = round(A*s + B).  Max rel err ~3.3%, applied
# to half the off-diagonal score tiles (DVE offload of the scalar
# engine's exp, which is phase B's per-block critical path).
SCH_A = 0.125 * 1.4426950408889634 * 128
SCH_B = 16256.0 - 5.6

_CACHE = {}


def _build_nc():
    import concourse.mybir as mybir
    from concourse import bacc, tile

    f32 = mybir.dt.float32
    bf16 = mybir.dt.bfloat16
    i16 = mybir.dt.int16
    AF = mybir.ActivationFunctionType
    OP = mybir.AluOpType

    nc = bacc.Bacc("TRN2", target_bir_lowering=False)

    # x pre-swizzled host-side to [n, c, p, f] so each n-group of 512
    # q-columns is one contiguous 1MB DMA (32 small strided DMAs starve
    # the projections: ~700ns/DMA of queue overhead halves the feed rate)
    xs = nc.dram_tensor("xs", [NCH, CHUNKS, 128, 512], bf16,
                        kind="ExternalInput")
    # weights pre-swizzled host-side to the exact SBUF tile layout so each
    # is one fully-contiguous DMA (per-chunk DMAs on the scalar queue get
    # interleaved with evacuation ACTIVATEs and starve the projections)
    wq = nc.dram_tensor("wq", [128, CHUNKS * HPC * DH], bf16,
                        kind="ExternalInput")
    wk = nc.dram_tensor("wk", [128, CHUNKS * HPC * DH], bf16,
                        kind="ExternalInput")
    wv = nc.dram_tensor("wv", [128, CHUNKS * HPC * DH], bf16,
                        kind="ExternalInput")
    wo = nc.dram_tensor("wo", [128, NPAIR * D], bf16, kind="ExternalInput")
    # per-pair q/k biases: [128, 4] cols = (q pair0, q pair1, k pair0, k pair1)
    bqk = nc.dram_tensor("bqk", [128, 2 * NPAIR], f32, kind="ExternalInput")
    # bv broadcast to all partitions host-side: [128, 256]
    bvb = nc.dram_tensor("bvb", [128, HPC * DH], bf16, kind="ExternalInput")
    out = nc.dram_tensor("out", [S, D], bf16, kind="ExternalOutput")

    with tile.TileContext(nc) as tc:
        with (
            tc.tile_pool(name="pers", bufs=1) as pers,
            tc.tile_pool(name="atnp", bufs=2) as atnp,
            tc.tile_pool(name="misc", bufs=1) as misc,
        ):
            qt = pers.tile([128, NPAIR, S], bf16)         # Q^T head pairs
            kt = pers.tile([128, NPAIR, S], bf16)         # K^T head pairs
            vsb = pers.tile([128, SBLK, HPC, DH + 1], bf16)  # [V | 1]
            wo_t = pers.tile([128, NPAIR, D], bf16)       # Wo head pairs
            ones2 = misc.tile([128, 2], bf16)   # [0 | 1] columns
            onef = misc.tile([128, 2], f32)
            onesrow = misc.tile([DH + 1, DH], bf16)  # row 64 = ones
            bias_t = misc.tile([128, 2 * NPAIR], f32)
            bvb_t = misc.tile([128, HPC * DH], bf16)
            dmy_w = misc.tile([128, 128], bf16)
            dmy_m = misc.tile([128, 512], bf16)
            # memsets first: the HAM warmup matmuls depend on them
            nc.gpsimd.memset(dmy_w[:], 0.25)
            nc.gpsimd.memset(dmy_m[:], 0.25)
            # small start-up DMAs ride the gpsimd queue (after the
            # memsets, which gate the HAM warmup) so sync can stream x
            # immediately
            nc.gpsimd.dma_start(bias_t[:], bqk[:])
            nc.gpsimd.dma_start(bvb_t[:], bvb[:])
            nc.gpsimd.memset(onef[:, 0:1], 0.0)
            nc.gpsimd.memset(onef[:, 1:2], 1.0)
            nc.vector.tensor_copy(ones2[:], onef[:])
            nc.vector.tensor_copy(
                onesrow[DH:DH + 1, :],
                onef[DH:DH + 1, 1:2].to_broadcast((1, DH)))
            # ones column of [V|1] for every (sblk, head)
            nc.vector.tensor_copy(
                vsb[:, :, :, DH:DH + 1],
                onef[:, 1:2].to_broadcast((128, SBLK, HPC, 1)))

            # ---------------- Phase A: projections ----------------
            ctxA = nc.named_scope("phaseA"); ctxA.__enter__()
            with (
                tc.tile_pool(name="wts", bufs=1) as wts,
                tc.tile_pool(name="psA", bufs=4, space="PSUM") as psA,
                tc.tile_pool(name="psV", bufs=2, space="PSUM") as psV,
                tc.tile_pool(name="psW", bufs=1, space="PSUM") as psW,
            ):
                dmy_ps = psW.tile([128, 512], f32, name="dmy_ps")
                xtr = wts.tile([128, CHUNKS, S], bf16)
                wq_t = wts.tile([128, CHUNKS, HPC * DH], bf16, tag="wq")
                wk_t = wts.tile([128, CHUNKS, HPC * DH], bf16, tag="wk")
                wv_t = wts.tile([128, CHUNKS, HPC * DH], bf16, tag="wv")

                # HAM warmers: no input deps, so the scheduler runs them
                # while the first x chunks are still in flight -- the PE
                # clock-gate is released (~3.4us of activity) before the
                # first real projection issues.
                # 256-col warmers: the p-state ramp needs CONTINUOUS
                # PE activity, not full-width streams -- narrower
                # matmuls span the same ~7us of DMA latency at half the
                # switching energy, so less of the HAM power budget is
                # spent before the real projections start
                for _ in range(WARMUP):
                    nc.tensor.matmul(dmy_ps[:, 0:256], dmy_w[:],
                                     dmy_m[:, 0:256],
                                     start=True, stop=True)

                # all inputs stream on the sync queue as big contiguous
                # DMAs, interleaved in consumption order: KT needs wk +
                # x(n=3) first, then wv (V), wq (QT), the rest of x, wo.
                def xdma(n, half=None):
                    sl = slice(512 * n, 512 * n + 512)
                    src = xs[n].rearrange("c p f -> p c f")
                    if half is None:
                        nc.sync.dma_start(xtr[:, :, sl], src)
                    else:
                        cs = slice(4 * half, 4 * half + 4)
                        nc.sync.dma_start(xtr[:, cs, sl], src[:, cs])

                nc.sync.dma_start(wk_t[:], wk[:])
                nc.sync.dma_start(wv_t[:], wv[:])
                xdma(3, 0)
                xdma(3, 1)
                nc.sync.dma_start(wq_t[:], wq[:])
                xdma(2)
                nc.sync.dma_start(wo_t[:], wo[:])
                xdma(1)
                xdma(0)

                # QT / KT: psum[128(2xdh), 512] accumulated over chunks.
                # Each QK group is interleaved matmul-by-matmul with one
                # V group: V's 256-col streams fully hide the QK
                # LDWEIGHTS and vice versa (a solo V group exposes ~half
                # its LDWEIGHTS time; zipped, neither does).
                def zip_group(dsts, p, n, sb, evac="scalar"):
                    w_tile = wq_t if dsts == "q" else wk_t
                    dst = qt if dsts == "q" else kt
                    bcol0 = 0 if dsts == "q" else NPAIR
                    ps = psA.tile([128, 512], f32, name="ps", tag="ps")
                    psv = psV.tile([128, HPC * DH], f32)
                    for c in range(CHUNKS):
                        nc.tensor.matmul(
                            ps[:],
                            w_tile[:, c, 128 * p:128 * p + 128],
                            xtr[:, c, 512 * n:512 * n + 512],
                            start=(c == 0), stop=(c == CHUNKS - 1))
                        nc.tensor.matmul(
                            psv[:],
                            xtr[:, c, 128 * sb:128 * sb + 128],
                            wv_t[:, c, :],
                            start=(c == 0), stop=(c == CHUNKS - 1))
                    # evacuate + add per-partition bias
                    sl = slice(512 * n, 512 * n + 512)
                    bias = bias_t[:, bcol0 + p:bcol0 + p + 1]
                    if evac == "scalar":
                        nc.scalar.activation(
                            dst[:, p, sl], ps[:], AF.Identity, bias=bias)
                    else:
                        nc.vector.tensor_tensor(
                            dst[:, p, sl], ps[:],
                            bias.to_broadcast((128, 512)), op=OP.add)
                    nc.vector.tensor_tensor(
                        vsb[:, sb, :, 0:DH],
                        psv[:].rearrange("p (h d) -> p h d", h=HPC),
                        bvb_t[:].rearrange("p (h d) -> p h d", h=HPC),
                        op=OP.add)

                # Phase B runs the q-chunks in REVERSE order (ch=3
                # first: it is all-diagonal and thin, so it hides in
                # phase A's tail while the kernel ends on the densest
                # chunk).  Emission order tracks consumption.
                for n in (3, 2, 1, 0):
                    qevac = "scalar" if n == 3 else "vector"
                    zip_group("k", 0, n, 4 * n + 0)
                    zip_group("k", 1, n, 4 * n + 1)
                    zip_group("q", 0, n, 4 * n + 2, evac=qevac)
                    zip_group("q", 1, n, 4 * n + 3, evac=qevac)

            ctxA.__exit__(None, None, None)
            # ------------- Phase B + fused C, per q-chunk -------------
            with (
                tc.tile_pool(name="pt", bufs=6) as ptp,
                tc.tile_pool(name="srow", bufs=2) as srowp,
                tc.tile_pool(name="rcp", bufs=2) as rcpp,
                tc.tile_pool(name="todd", bufs=2) as toddp,
                tc.tile_pool(name="ob", bufs=4) as obp,
                tc.tile_pool(name="psS", bufs=2, space="PSUM") as psS,
                tc.tile_pool(name="psAt", bufs=1, space="PSUM") as psAt,
            ):
                psa_tiles = {}   # ch -> [psa tile per head]
                atn_tiles = {}   # ch -> atn tile

                def ensure_psa(ch):
                    if ch not in psa_tiles:
                        psa_tiles[ch] = [
                            psAt.tile([DH + 1, 512], f32, tag=f"psa{h}",
                                      name=f"psa{h}")
                            for h in range(HPC)]

                def emit_scores(ch, j):
                    ensure_psa(ch)
                    d = j - 4 * ch
                    W = 128 * (d + 1) if d < 4 else 512
                    pts = []
                    for pair in range(NPAIR):
                        # the two heads of the pair contract disjoint
                        # 64-partition ranges -> row-tiled concurrent
                        # matmuls into the two banks of one PSUM tile
                        pss = psS.tile([128, 2, 512], f32, tag="s",
                                       name="pss")
                        for half in range(2):
                            rows = slice(64 * half, 64 * half + 64)
                            nc.tensor.matmul(
                                pss[:, half, 0:W],
                                kt[rows, pair, 128 * j:128 * j + 128],
                                qt[rows, pair, 512 * ch:512 * ch + W],
                                start=True, stop=True)
                        pt = ptp.tile([128, 2, 512], bf16)
                        if d >= 4 and (j + pair) % 2 == 0:
                            # DVE fast-exp offload (Schraudolph int16
                            # bit trick) -- the scalar engine's exp is
                            # phase B's per-block critical path, so half
                            # the off-diagonal tiles exp on the DVE.
                            # (Splitting every tile across BOTH engines
                            # measured ~4us slower: the doubled
                            # instruction count adds ~167ns of semaphore
                            # overhead per op on both queues.)
                            nc.vector.tensor_scalar(
                                pt[:, :, 0:W].bitcast(i16),
                                pss[:, :, 0:W],
                                SCH_A, SCH_B, op0=OP.mult, op1=OP.add)
                        else:
                            nc.scalar.activation(pt[:, :, 0:W],
                                                 pss[:, :, 0:W],
                                                 AF.Exp, scale=0.125)
                        if d < 4:
                            # zero where k <= q, i.e. keep where
                            # 128d + k_local - q > 0, via an affine
                            # predicate on the gpsimd engine (frees the
                            # DVE, no mask tile or DMA needed)
                            nc.gpsimd.affine_select(
                                pt[:, :, 0:W], pt[:, :, 0:W],
                                pattern=[[0, 2], [-1, W]],
                                compare_op=OP.is_gt, fill=0.0,
                                base=128 * d, channel_multiplier=1)
                        pts.append(pt)
                    return pts

                started = set()

                def emit_attn(ch, j, pts):
                    # emitted one j behind the scores (software pipeline:
                    # the PE queue is strict FIFO, so scores(j+1) must sit
                    # ahead of attn(j) -- attn waits on exp(j), and the PE
                    # streams scores(j+1) during that wait)
                    psa = psa_tiles[ch]
                    d = j - 4 * ch
                    W = 128 * (d + 1) if d < 4 else 512
                    first = ch not in started
                    started.add(ch)
                    last = (j == 4 * ch + 3) and ch < 3
                    # head order matches the norm order (odd heads
                    # first): their psa accumulators stop first, letting
                    # the norm chains start earliest
                    for h in (1, 3, 0, 2):
                        nc.tensor.matmul(
                            psa[h][:, 0:W], vsb[:, j, h, :],
                            pts[h // 2][:, h % 2, 0:W],
                            start=first, stop=last,
                            skip_group_check=(ch == 3))

                def emit_norm(ch, last=False):
                    psa = psa_tiles[ch]
                    if ch == 3:
                        # last global row q=2047 is fully masked; its
                        # exact value is recomputed on the host.  Keep
                        # column 511's denominator finite (one [0|1]-
                        # column matmul) to avoid Inf/NaN noise.
                        for h in range(HPC):
                            nc.tensor.matmul(
                                psa[h][:, 510:512],
                                vsb[:, 0, h, :], ones2[:],
                                start=False, stop=True)
                    # normalize: attn^T rows / sums row.  Broadcast the
                    # sums row via a K=1 ones matmul, 64-lane approx
                    # reciprocal, then multiply.  Odd heads go through a
                    # SBUF tile and a partition-shifting DMA into rows
                    # 64:128 of the pair tile so phase C contracts K=128.
                    # normalize: attn^T rows / sums row.  The sums row is
                    # copied to SBUF (scalar), broadcast to 64 partitions
                    # with a K=1 ones matmul, reciprocal'd and multiplied
                    # on the DVE.  Odd heads go through a SBUF tile and a
                    # partition-shifting DMA into rows 64:128 of the pair
                    # tile so phase C contracts K=128.
                    atn = atnp.tile([128, NPAIR, 512], bf16)
                    atn_tiles[ch] = atn
                    # odd heads first: their partition-shifting bounce
                    # DMAs (~2us completion latency each) start early and
                    # overlap the even heads' norm chains.  The LAST
                    # chunk's norm is split into two q-halves so phase C
                    # (which consumes 128-q blocks) starts ~2us earlier
                    # in the kernel tail; steady-state chunks stay whole
                    # (fewer instructions, the latency is hidden there).
                    halves = ((0, 256), (256, 512)) if last else ((0, 512),)
                    for q0, q1 in halves:
                        qs, qw = slice(q0, q1), q1 - q0
                        for h in (1, 3, 0, 2):
                            pair, half = h // 2, h % 2
                            srow = srowp.tile([DH + 1, 512], bf16)
                            # sums-row copy on scalar (PSUM-near, idle
                            # during norm): keeps the DVE free for the
                            # reciprocal+multiply chain in the tail
                            nc.scalar.copy(srow[DH:DH + 1, qs],
                                           psa[h][DH:DH + 1, qs])
                            bcs = psS.tile([64, 512], f32, tag="s",
                                           name="bcs")
                            # bf16 K=1 broadcast matmul: sums only need
                            # bf16 precision (0.4% on the denominator,
                            # well under the error budget) and bf16
                            # streams 2-3x faster than the fp32 paths
                            nc.tensor.matmul(bcs[:, 0:qw],
                                             onesrow[DH:DH + 1, :],
                                             srow[DH:DH + 1, qs],
                                             start=True, stop=True)
                            rcp = rcpp.tile([64, 512], f32)
                            nc.vector.reciprocal_approx_fast(
                                rcp[:, 0:qw], bcs[:, 0:qw])
                            if half == 0:
                                nc.vector.tensor_tensor(
                                    atn[0:64, pair, qs],
                                    psa[h][0:DH, qs],
                                    rcp[:, 0:qw], op=OP.mult)
                            else:
                                todd = toddp.tile([64, 512], bf16)
                                nc.vector.tensor_tensor(
                                    todd[:, 0:qw], psa[h][0:DH, qs],
                                    rcp[:, 0:qw], op=OP.mult)
                                # gpsimd queue: no out contention
                                nc.gpsimd.dma_start(
                                    atn[64:128, pair, qs],
                                    todd[:, 0:qw])

                def emit_c(ch):
                    # fused phase C for this chunk's 4 s-blocks;
                    # evacuations alternate scalar/DVE so the 2-slot psum
                    # pool never rate-limits the PE, and out-writes
                    # alternate sync/gpsimd queues
                    atn = atn_tiles[ch]
                    for k in range(4):
                        sb = 4 * ch + k
                        # both n-halves fill one 2-bank psum tile: twice
                        # the pipeline depth from the 2-slot pool, and
                        # the two evacuations run on scalar+DVE in
                        # parallel
                        ps = psS.tile([128, 2, 512], f32, tag="s",
                                      name="psO")
                        for n in range(2):
                            for p in range(NPAIR):
                                nc.tensor.matmul(
                                    ps[:, n, :],
                                    atn[:, p, 128 * k:128 * k + 128],
                                    wo_t[:, p, 512 * n:512 * n + 512],
                                    start=(p == 0), stop=(p == NPAIR - 1))
                        ob = obp.tile([128, 2, 512], bf16)
                        nc.scalar.copy(ob[:, 0, :], ps[:, 0, :])
                        nc.vector.tensor_copy(ob[:, 1, :], ps[:, 1, :])
                        nc.sync.dma_start(
                            out[128 * sb:128 * sb + 128, 0:512],
                            ob[:, 0, :])
                        nc.gpsimd.dma_start(
                            out[128 * sb:128 * sb + 128, 512:1024],
                            ob[:, 1, :])

                # Software-pipelined emission: chunks run 3,2,1,0; each
                # chunk runs its off-diagonal j-pairs (fp8 DoubleRow)
                # first and the 4 diagonal singles last.  Items stolen
                # across a chunk boundary are emitted around norm/C of
                # the previous chunk to keep every engine busy there.
                # Attn lags the scores by up to 2 items: the pair-1 exp
                # chain (DVE schraudolph -> gpsimd fp8 cast) is two
                # engine hops long, so one item of PE work is not enough
                # to cover it.
                seq = [3, 2, 1, 0]

                def items_of(ch):
                    its = [("j", ch, j)
                           for j in (list(range(4 * ch + 4, SBLK))
                                     + list(range(4 * ch, 4 * ch + 4)))]
                    return its + [("norm", ch), ("c", ch)]

                work = {ch: items_of(ch) for ch in seq}
                order = []
                for i, ch in enumerate(seq):
                    items = work[ch]
                    nxt = work[seq[i + 1]] if i + 1 < len(seq) else []
                    pre = nxt[:4] if nxt else []
                    del nxt[:4]
                    order.extend(items[:-2])
                    order.extend(pre[:1])
                    order.append(items[-2])       # norm
                    order.extend(pre[1:4])
                    order.append(items[-1])       # C
                # attn lags the scores by up to 2 j-items: the
                # exp/mask chains span 2-3 engine hops, so one item of
                # PE work is not always enough to cover them
                pending = []   # (ch, j, pts) whose attn is not yet out
                for item in order:
                    if item[0] == "j":
                        ch, j = item[1], item[2]
                        pts = emit_scores(ch, j)
                        pending.append((ch, j, pts))
                        while len(pending) > 2:
                            emit_attn(*pending.pop(0))
                    elif item[0] == "norm":
                        ch = item[1]
                        mine = [r for r in pending if r[0] == ch]
                        pending = [r for r in pending if r[0] != ch]
                        for r in mine:
                            emit_attn(*r)
                        emit_norm(ch, last=(ch == seq[-1]))
                    else:
                        emit_c(item[1])
                for r in pending:
                    emit_attn(*r)

    nc.finalize()
    return nc


def _prep_in_maps(inputs, Wq, bq, Wk, bk, Wv, bv, Wo, bo):
    import ml_dtypes
    bf16 = ml_dtypes.bfloat16

    in_maps = []
    # [D, S] -> [n, c, p, f] contiguous n-groups (see kernel docstring)
    xTs = [np.ascontiguousarray(
        inputs[b].T.astype(bf16).reshape(CHUNKS, 128, NCH, 512)
        .transpose(2, 0, 1, 3)) for b in range(B)]

    def _wswiz(w):
        # [D, M] -> [128, CHUNKS*M]: SBUF tile layout, fully contiguous
        m = w.shape[1]
        return np.ascontiguousarray(
            w.astype(bf16).reshape(CHUNKS, 128, m)
            .transpose(1, 0, 2).reshape(128, CHUNKS * m))

    for core in range(NCORES):
        b = core // (NCORES // B)
        g = core % (NCORES // B)
        cols = slice(g * HPC * DH, (g + 1) * HPC * DH)
        bq_c = bq[cols].reshape(NPAIR, 128).T          # [128, 2]
        bk_c = bk[cols].reshape(NPAIR, 128).T
        bqk_c = np.ascontiguousarray(
            np.concatenate([bq_c, bk_c], axis=1), dtype=np.float32)
        bvb_c = np.ascontiguousarray(
            np.broadcast_to(bv[cols][None, :], (128, HPC * DH))).astype(bf16)
        wo_c = np.ascontiguousarray(
            Wo[cols, :].astype(bf16).reshape(NPAIR, 128, D)
            .transpose(1, 0, 2).reshape(128, NPAIR * D))
        in_maps.append({
            "xs": xTs[b],
            "wq": _wswiz(np.ascontiguousarray(Wq[:, cols])),
            "wk": _wswiz(np.ascontiguousarray(Wk[:, cols])),
            "wv": _wswiz(np.ascontiguousarray(Wv[:, cols])),
            "wo": wo_c,
            "bqk": bqk_c,
            "bvb": bvb_c,
        })
    return in_maps


def kernel(inputs, Wq, bq, Wk, bk, Wv, bv, Wo, bo, _want_results=False,
           **_run_kwargs):
    from concourse.bass_utils import run_bass_kernel_spmd

    inputs = np.asarray(inputs, dtype=np.float32)
    Wq, bq = np.asarray(Wq, np.float32), np.asarray(bq, np.float32)
    Wk, bk = np.asarray(Wk, np.float32), np.asarray(bk, np.float32)
    Wv, bv = np.asarray(Wv, np.float32), np.asarray(bv, np.float32)
    Wo, bo = np.asarray(Wo, np.float32), np.asarray(bo, np.float32)

    if "nc" not in _CACHE:
        _CACHE["nc"] = _build_nc()
    nc = _CACHE["nc"]

    in_maps = _prep_in_maps(inputs, Wq, bq, Wk, bk, Wv, bv, Wo, bo)
    res = run_bass_kernel_spmd(nc, in_maps, core_ids=list(range(NCORES)),
                               **_run_kwargs)

    out = np.zeros((B, S, D), dtype=np.float32)
    for core in range(NCORES):
        b = core // (NCORES // B)
        out[b] += res.results[core]["out"].astype(np.float32)
    out += bo[None, None, :]
    # exact last row (fully masked -> uniform attention = mean(V) @ Wo)
    for b in range(B):
        v_mean = inputs[b].mean(axis=0) @ Wv + bv
        out[b, S - 1, :] = v_mean @ Wo + bo
    if _want_results:
        return out, res
    return out



# revision 65
# speedup vs baseline: 1.0219x; 1.0219x over previous
"""Causal self-attention (with the reference's inverted mask) on 8 TRN2
NeuronCores.

Problem (hardcoded): B=2, S=2048, D=1024, H=16 heads, head_dim=64, fp32.
  q/k/v = x @ W* + b*;  score = q k^T / 8;  score += tril(ones)*(-1e9)
  (inverted causal mask: the LOWER triangle incl. diagonal is masked, so
  softmax attends strictly to k > q; row q=S-1 is fully masked and its
  softmax is exactly uniform);  out = softmax(score) @ v @ Wo + bo.

Sharding: core c handles batch b = c//4 and heads [4*(c%4), 4*(c%4)+4).
Each core computes a partial output (its 4 heads' slice of attn @ Wo);
the host sums 4 partials per batch and adds bo.

v3 kernel (all matmuls bf16; inputs pre-swizzled host-side).  The
device is power-limited: HAM duty-cycles the PE to 4/8 under sustained
load, so the design minimizes total PE streaming work and keeps every
engine's queue busy rather than chasing per-matmul peak rates (fp8
DoubleRow measured SLOWER end-to-end: 2 multiplies/cell doubles the
switching power and triples the throttle time).

  Inputs: x arrives as 4 contiguous 1MB DMAs (one per 512-column
    n-slice, n=3 first so phase B's first q-chunk can start early);
    each weight is one contiguous DMA in SBUF tile layout.  All input
    DMAs ride the sync queue in consumption order; small tiles ride
    gpsimd.  A short burst of dummy matmuls covers the first DMAs'
    latency and warms the PE p-state ramp.
  Phase A: QT/KT = W^T x^T in [dh, s] layout (head pairs packed to 128
    partitions), V in [s, dh] layout with an extra ones column per head
    ([V | 1]) so one matmul later yields both the attn numerator and
    the softmax denominator.  Each 512-col QK group is interleaved
    matmul-by-matmul with a 256-col V group so neither side's
    LDWEIGHTS is exposed.
  Phase B (per q-chunk of 512): scores computed TRANSPOSED,
    s^T[k, q] = K^T Q per (head, k-block j).  The two heads of a pair
    contract only K=64 partitions each, so they run as row-tiled
    CONCURRENT matmuls writing the two banks of one [128, 2, 512] PSUM
    tile -- score streaming cost is halved vs a zero-padded K=128
    contraction, and one strided exp per (j, pair) covers both heads.
    p^T = exp(s^T/8) in bf16: off-diagonal tiles split between scalar
    ACT (exact) and DVE Schraudolph (int16 bit trick, ~3.3 percent);
    diagonal tiles exp on scalar and get an affine_select on gpsimd
    (zero where k <= q, no mask tile needed), narrowed to the 128(d+1)
    live columns.  attn^T[dh|sum, q] accumulates matmul([V|1], p^T)
    over j in PSUM per head; attn emission lags the scores by 2 j-items
    (software pipeline over the strict-FIFO PE queue).
    The globally-masked last row (q=2047) is recomputed exactly on the
    host; N=2 [0|1]-column matmuls keep its on-chip denominator finite.
    Normalization per head: sums row -> bf16 SBUF (scalar), K=1 ones
    matmul broadcasts it to 64 partitions, DVE fast reciprocal +
    multiply; odd heads bounce through SBUF + a partition-shifting
    gpsimd DMA into rows 64:128 of the pair tile so phase C contracts
    K=128.  Odd heads run first so the ~2us bounce latency overlaps
    the even heads' chains; the last chunk's norm is split into two
    q-halves so phase C can start while the second half normalizes.
  Phase C (fused per q-chunk): out_partial = attn^T.T @ Wo-rows, heads
    packed in pairs so the contraction runs K=128.  Both 512-col
    output halves of an s-block fill one 2-bank PSUM tile, evacuated
    in parallel on scalar+DVE, written back on sync+gpsimd queues.
    Cross-chunk: 4 score items of the next chunk are emitted around
    the current chunk's norm/C to keep the PE busy through the
    boundary.  PSUM budget: 4 banks attn accumulators + one 2-slot
    pool (4 banks) shared by score tiles, sum-broadcast tiles and
    phase C outputs.
"""

import numpy as np

B, S, D, H, DH = 2, 2048, 1024, 16, 64
HPC = 4                 # heads per core
NCORES = 8
NPAIR = HPC // 2        # head pairs per core (2)
SBLK = S // 128         # 16 s/k blocks
NCH = S // 512          # 4 q-chunks of 512
CHUNKS = D // 128       # 8 contraction chunks of the model dim
WARMUP = 40             # HAM-warming dummy matmuls at kernel start
# Schraudolph fast-exp constants: exp(s/8) ~= bf16(bits = A*s + B) with
# bits computed as int16 = round(A*s + B).  Max rel err ~3.3%, applied
# to half the off-diagonal score tiles (DVE offload of the scalar
# engine's exp, which is phase B's per-block critical path).
SCH_A = 0.125 * 1.4426950408889634 * 128
SCH_B = 16256.0 - 5.6

_CACHE = {}


def _build_nc():
    import concourse.mybir as mybir
    from concourse import bacc, tile

    f32 = mybir.dt.float32
    bf16 = mybir.dt.bfloat16
    i16 = mybir.dt.int16
    AF = mybir.ActivationFunctionType
    OP = mybir.AluOpType

    nc = bacc.Bacc("TRN2", target_bir_lowering=False)

    # x pre-swizzled host-side to [n, c, p, f] so each n-group of 512
    # q-columns is one contiguous 1MB DMA (32 small strided DMAs starve
    # the projections: ~700ns/DMA of queue overhead halves the feed rate)
    xs = nc.dram_tensor("xs", [NCH, CHUNKS, 128, 512], bf16,
                        kind="ExternalInput")
    # weights pre-swizzled host-side to the exact SBUF tile layout so each
    # is one fully-contiguous DMA (per-chunk DMAs on the scalar queue get
    # interleaved with evacuation ACTIVATEs and starve the projections)
    wq = nc.dram_tensor("wq", [128, CHUNKS * HPC * DH], bf16,
                        kind="ExternalInput")
    wk = nc.dram_tensor("wk", [128, CHUNKS * HPC * DH], bf16,
                        kind="ExternalInput")
    wv = nc.dram_tensor("wv", [128, CHUNKS * HPC * DH], bf16,
                        kind="ExternalInput")
    wo = nc.dram_tensor("wo", [128, NPAIR * D], bf16, kind="ExternalInput")
    # per-pair q/k biases: [128, 4] cols = (q pair0, q pair1, k pair0, k pair1)
    bqk = nc.dram_tensor("bqk", [128, 2 * NPAIR], f32, kind="ExternalInput")
    # bv broadcast to all partitions host-side: [128, 256]
    bvb = nc.dram_tensor("bvb", [128, HPC * DH], bf16, kind="ExternalInput")
    out = nc.dram_tensor("out", [S, D], bf16, kind="ExternalOutput")

    with tile.TileContext(nc) as tc:
        with (
            tc.tile_pool(name="pers", bufs=1) as pers,
            tc.tile_pool(name="atnp", bufs=2) as atnp,
            tc.tile_pool(name="misc", bufs=1) as misc,
        ):
            qt = pers.tile([128, NPAIR, S], bf16)         # Q^T head pairs
            kt = pers.tile([128, NPAIR, S], bf16)         # K^T head pairs
            vsb = pers.tile([128, SBLK, HPC, DH + 1], bf16)  # [V | 1]
            wo_t = pers.tile([128, NPAIR, D], bf16)       # Wo head pairs
            ones2 = misc.tile([128, 2], bf16)   # [0 | 1] columns
            onef = misc.tile([128, 2], f32)
            onesrow = misc.tile([DH + 1, DH], bf16)  # row 64 = ones
            bias_t = misc.tile([128, 2 * NPAIR], f32)
            bvb_t = misc.tile([128, HPC * DH], bf16)
            dmy_w = misc.tile([128, 128], bf16)
            dmy_m = misc.tile([128, 512], bf16)
            # memsets first: the HAM warmup matmuls depend on them
            nc.gpsimd.memset(dmy_w[:], 0.25)
            nc.gpsimd.memset(dmy_m[:], 0.25)
            # small start-up DMAs ride the gpsimd queue (after the
            # memsets, which gate the HAM warmup) so sync can stream x
            # immediately
            nc.gpsimd.dma_start(bias_t[:], bqk[:])
            nc.gpsimd.dma_start(bvb_t[:], bvb[:])
            nc.gpsimd.memset(onef[:, 0:1], 0.0)
            nc.gpsimd.memset(onef[:, 1:2], 1.0)
            nc.vector.tensor_copy(ones2[:], onef[:])
            nc.vector.tensor_copy(
                onesrow[DH:DH + 1, :],
                onef[DH:DH + 1, 1:2].to_broadcast((1, DH)))
            # ones column of [V|1] for every (sblk, head)
            nc.vector.tensor_copy(
                vsb[:, :, :, DH:DH + 1],
                onef[:, 1:2].to_broadcast((128, SBLK, HPC, 1)))

            # ---------------- Phase A: projections ----------------
            ctxA = nc.named_scope("phaseA"); ctxA.__enter__()
            with (
                tc.tile_pool(name="wts", bufs=1) as wts,
                tc.tile_pool(name="psA", bufs=4, space="PSUM") as psA,
                tc.tile_pool(name="psV", bufs=2, space="PSUM") as psV,
                tc.tile_pool(name="psW", bufs=1, space="PSUM") as psW,
            ):
                dmy_ps = psW.tile([128, 512], f32, name="dmy_ps")
                xtr = wts.tile([128, CHUNKS, S], bf16)
                wq_t = wts.tile([128, CHUNKS, HPC * DH], bf16, tag="wq")
                wk_t = wts.tile([128, CHUNKS, HPC * DH], bf16, tag="wk")
                wv_t = wts.tile([128, CHUNKS, HPC * DH], bf16, tag="wv")

                # HAM warmers: no input deps, so the scheduler runs them
                # while the first x chunks are still in flight -- the PE
                # clock-gate is released (~3.4us of activity) before the
                # first real projection issues.
                # 256-col warmers: the p-state ramp needs CONTINUOUS
                # PE activity, not full-width streams -- narrower
                # matmuls span the same ~7us of DMA latency at half the
                # switching energy, so less of the HAM power budget is
                # spent before the real projections start
                for _ in range(WARMUP):
                    nc.tensor.matmul(dmy_ps[:, 0:256], dmy_w[:],
                                     dmy_m[:, 0:256],
                                     start=True, stop=True)

                # all inputs stream on the sync queue as big contiguous
                # DMAs, interleaved in consumption order: KT needs wk +
                # x(n=3) first, then wv (V), wq (QT), the rest of x, wo.
                def xdma(n, half=None):
                    sl = slice(512 * n, 512 * n + 512)
                    src = xs[n].rearrange("c p f -> p c f")
                    if half is None:
                        nc.sync.dma_start(xtr[:, :, sl], src)
                    else:
                        cs = slice(4 * half, 4 * half + 4)
                        nc.sync.dma_start(xtr[:, cs, sl], src[:, cs])

                nc.sync.dma_start(wk_t[:], wk[:])
                nc.sync.dma_start(wv_t[:], wv[:])
                xdma(3, 0)
                xdma(3, 1)
                nc.sync.dma_start(wq_t[:], wq[:])
                xdma(2)
                nc.sync.dma_start(wo_t[:], wo[:])
                xdma(1)
                xdma(0)

                # QT / KT: psum[128(2xdh), 512] accumulated over chunks.
                # Each QK group is interleaved matmul-by-matmul with one
                # V group: V's 256-col streams fully hide the QK
                # LDWEIGHTS and vice versa (a solo V group exposes ~half
                # its LDWEIGHTS time; zipped, neither does).
                def zip_group(dsts, p, n, sb, evac="scalar"):
                    w_tile = wq_t if dsts == "q" else wk_t
                    dst = qt if dsts == "q" else kt
                    bcol0 = 0 if dsts == "q" else NPAIR
                    ps = psA.tile([128, 512], f32, name="ps", tag="ps")
                    psv = psV.tile([128, HPC * DH], f32)
                    for c in range(CHUNKS):
                        nc.tensor.matmul(
                            ps[:],
                            w_tile[:, c, 128 * p:128 * p + 128],
                            xtr[:, c, 512 * n:512 * n + 512],
                            start=(c == 0), stop=(c == CHUNKS - 1))
                        nc.tensor.matmul(
                            psv[:],
                            xtr[:, c, 128 * sb:128 * sb + 128],
                            wv_t[:, c, :],
                            start=(c == 0), stop=(c == CHUNKS - 1))
                    # evacuate + add per-partition bias
                    sl = slice(512 * n, 512 * n + 512)
                    bias = bias_t[:, bcol0 + p:bcol0 + p + 1]
                    if evac == "scalar":
                        nc.scalar.activation(
                            dst[:, p, sl], ps[:], AF.Identity, bias=bias)
                    else:
                        nc.vector.tensor_tensor(
                            dst[:, p, sl], ps[:],
                            bias.to_broadcast((128, 512)), op=OP.add)
                    nc.vector.tensor_tensor(
                        vsb[:, sb, :, 0:DH],
                        psv[:].rearrange("p (h d) -> p h d", h=HPC),
                        bvb_t[:].rearrange("p (h d) -> p h d", h=HPC),
                        op=OP.add)

                # Phase B runs the q-chunks in REVERSE order (ch=3
                # first: it is all-diagonal and thin, so it hides in
                # phase A's tail while the kernel ends on the densest
                # chunk).  Emission order tracks consumption.
                for n in (3, 2, 1, 0):
                    qevac = "scalar" if n == 3 else "vector"
                    zip_group("k", 0, n, 4 * n + 0)
                    zip_group("k", 1, n, 4 * n + 1)
                    zip_group("q", 0, n, 4 * n + 2, evac=qevac)
                    zip_group("q", 1, n, 4 * n + 3, evac=qevac)

            ctxA.__exit__(None, None, None)
            # ------------- Phase B + fused C, per q-chunk -------------
            with (
                tc.tile_pool(name="pt", bufs=6) as ptp,
                tc.tile_pool(name="srow", bufs=2) as srowp,
                tc.tile_pool(name="rcp", bufs=2) as rcpp,
                tc.tile_pool(name="todd", bufs=2) as toddp,
                tc.tile_pool(name="ob", bufs=4) as obp,
                tc.tile_pool(name="psS", bufs=2, space="PSUM") as psS,
                tc.tile_pool(name="psAt", bufs=1, space="PSUM") as psAt,
            ):
                psa_tiles = {}   # ch -> [psa tile per head]
                atn_tiles = {}   # ch -> atn tile

                def ensure_psa(ch):
                    if ch not in psa_tiles:
                        psa_tiles[ch] = [
                            psAt.tile([DH + 1, 512], f32, tag=f"psa{h}",
                                      name=f"psa{h}")
                            for h in range(HPC)]

                def emit_scores(ch, j):
                    ensure_psa(ch)
                    d = j - 4 * ch
                    W = 128 * (d + 1) if d < 4 else 512
                    pts = []
                    for pair in range(NPAIR):
                        # the two heads of the pair contract disjoint
                        # 64-partition ranges -> row-tiled concurrent
                        # matmuls into the two banks of one PSUM tile
                        pss = psS.tile([128, 2, 512], f32, tag="s",
                                       name="pss")
                        for half in range(2):
                            rows = slice(64 * half, 64 * half + 64)
                            nc.tensor.matmul(
                                pss[:, half, 0:W],
                                kt[rows, pair, 128 * j:128 * j + 128],
                                qt[rows, pair, 512 * ch:512 * ch + W],
                                start=True, stop=True)
                        pt = ptp.tile([128, 2, 512], bf16)
                        if d >= 4 and (j + pair) % 2 == 0:
                            # DVE fast-exp offload (Schraudolph int16
                            # bit trick) -- the scalar engine's exp is
                            # phase B's per-block critical path, so half
                            # the off-diagonal tiles exp on the DVE.
                            # (Splitting every tile across BOTH engines
                            # measured ~4us slower: the doubled
                            # instruction count adds ~167ns of semaphore
                            # overhead per op on both queues.)
                            nc.vector.tensor_scalar(
                                pt[:, :, 0:W].bitcast(i16),
                                pss[:, :, 0:W],
                                SCH_A, SCH_B, op0=OP.mult, op1=OP.add)
                        else:
                            nc.scalar.activation(pt[:, :, 0:W],
                                                 pss[:, :, 0:W],
                                                 AF.Exp, scale=0.125)
                        if d < 4:
                            # zero where k <= q, i.e. keep where
                            # 128d + k_local - q > 0, via an affine
                            # predicate on the gpsimd engine (frees the
                            # DVE, no mask tile or DMA needed)
                            nc.gpsimd.affine_select(
                                pt[:, :, 0:W], pt[:, :, 0:W],
                                pattern=[[0, 2], [-1, W]],
                                compare_op=OP.is_gt, fill=0.0,
                                base=128 * d, channel_multiplier=1)
                        pts.append(pt)
                    return pts

                started = set()

                def emit_attn(ch, j, pts):
                    # emitted one j behind the scores (software pipeline:
                    # the PE queue is strict FIFO, so scores(j+1) must sit
                    # ahead of attn(j) -- attn waits on exp(j), and the PE
                    # streams scores(j+1) during that wait)
                    psa = psa_tiles[ch]
                    d = j - 4 * ch
                    W = 128 * (d + 1) if d < 4 else 512
                    first = ch not in started
                    started.add(ch)
                    last = (j == 4 * ch + 3) and ch < 3
                    # head order matches the norm order (odd heads
                    # first): their psa accumulators stop first, letting
                    # the norm chains start earliest
                    for h in (1, 3, 0, 2):
                        nc.tensor.matmul(
                            psa[h][:, 0:W], vsb[:, j, h, :],
                            pts[h // 2][:, h % 2, 0:W],
                            start=first, stop=last,
                            skip_group_check=(ch == 3))

                def emit_norm(ch, last=False):
                    psa = psa_tiles[ch]
                    if ch == 3:
                        # last global row q=2047 is fully masked; its
                        # exact value is recomputed on the host.  Keep
                        # column 511's denominator finite (one [0|1]-
                        # column matmul) to avoid Inf/NaN noise.
                        for h in range(HPC):
                            nc.tensor.matmul(
                                psa[h][:, 510:512],
                                vsb[:, 0, h, :], ones2[:],
                                start=False, stop=True)
                    # normalize: attn^T rows / sums row.  Broadcast the
                    # sums row via a K=1 ones matmul, 64-lane approx
                    # reciprocal, then multiply.  Odd heads go through a
                    # SBUF tile and a partition-shifting DMA into rows
                    # 64:128 of the pair tile so phase C contracts K=128.
                    # normalize: attn^T rows / sums row.  The sums row is
                    # copied to SBUF (scalar), broadcast to 64 partitions
                    # with a K=1 ones matmul, reciprocal'd and multiplied
                    # on the DVE.  Odd heads go through a SBUF tile and a
                    # partition-shifting DMA into rows 64:128 of the pair
                    # tile so phase C contracts K=128.
                    atn = atnp.tile([128, NPAIR, 512], bf16)
                    atn_tiles[ch] = atn
                    # odd heads first: their partition-shifting bounce
                    # DMAs (~2us completion latency each) start early and
                    # overlap the even heads' norm chains.  The LAST
                    # chunk's norm is split into two q-halves so phase C
                    # (which consumes 128-q blocks) starts ~2us earlier
                    # in the kernel tail; steady-state chunks stay whole
                    # (fewer instructions, the latency is hidden there).
                    halves = ((0, 256), (256, 512)) if last else ((0, 512),)
                    for q0, q1 in halves:
                        qs, qw = slice(q0, q1), q1 - q0
                        for h in (1, 3, 0, 2):
                            pair, half = h // 2, h % 2
                            srow = srowp.tile([DH + 1, 512], bf16)
                            # sums-row copy on scalar (PSUM-near, idle
                            # during norm): keeps the DVE free for the
                            # reciprocal+multiply chain in the tail
                            nc.scalar.copy(srow[DH:DH + 1, qs],
                                           psa[h][DH:DH + 1, qs])
                            bcs = psS.tile([64, 512], f32, tag="s",
                                           name="bcs")
                            # bf16 K=1 broadcast matmul: sums only need
                            # bf16 precision (0.4% on the denominator,
                            # well under the error budget) and bf16
                            # streams 2-3x faster than the fp32 paths
                            nc.tensor.matmul(bcs[:, 0:qw],
                                             onesrow[DH:DH + 1, :],
                                             srow[DH:DH + 1, qs],
                                             start=True, stop=True)
                            rcp = rcpp.tile([64, 512], f32)
                            nc.vector.reciprocal_approx_fast(
                                rcp[:, 0:qw], bcs[:, 0:qw])
                            if half == 0:
                                nc.vector.tensor_tensor(
                                    atn[0:64, pair, qs],
                                    psa[h][0:DH, qs],
                                    rcp[:, 0:qw], op=OP.mult)
                            else:
                                todd = toddp.tile([64, 512], bf16)
                                nc.vector.tensor_tensor(
                                    todd[:, 0:qw], psa[h][0:DH, qs],
                                    rcp[:, 0:qw], op=OP.mult)
                                # gpsimd queue: no out contention
                                nc.gpsimd.dma_start(
                                    atn[64:128, pair, qs],
                                    todd[:, 0:qw])

                def emit_c(ch):
                    # fused phase C for this chunk's 4 s-blocks;
                    # evacuations alternate scalar/DVE so the 2-slot psum
                    # pool never rate-limits the PE, and out-writes
                    # alternate sync/gpsimd queues
                    atn = atn_tiles[ch]
                    for k in range(4):
                        sb = 4 * ch + k
                        # both n-halves fill one 2-bank psum tile: twice
                        # the pipeline depth from the 2-slot pool, and
                        # the two evacuations run on scalar+DVE in
                        # parallel
                        ps = psS.tile([128, 2, 512], f32, tag="s",
                                      name="psO")
                        for n in range(2):
                            for p in range(NPAIR):
                                nc.tensor.matmul(
                                    ps[:, n, :],
                                    atn[:, p, 128 * k:128 * k + 128],
                                    wo_t[:, p, 512 * n:512 * n + 512],
                                    start=(p == 0), stop=(p == NPAIR - 1))
                        ob = obp.tile([128, 2, 512], bf16)
                        nc.scalar.copy(ob[:, 0, :], ps[:, 0, :])
                        nc.vector.tensor_copy(ob[:, 1, :], ps[:, 1, :])
                        nc.sync.dma_start(
                            out[128 * sb:128 * sb + 128, 0:512],
                            ob[:, 0, :])
                        nc.gpsimd.dma_start(
                            out[128 * sb:128 * sb + 128, 512:1024],
                            ob[:, 1, :])

                # Software-pipelined emission: chunks run 3,2,1,0; each
                # chunk runs its off-diagonal j-pairs (fp8 DoubleRow)
                # first and the 4 diagonal singles last.  Items stolen
                # across a chunk boundary are emitted around norm/C of
                # the previous chunk to keep every engine busy there.
                # Attn lags the scores by up to 2 items: the pair-1 exp
                # chain (DVE schraudolph -> gpsimd fp8 cast) is two
                # engine hops long, so one item of PE work is not enough
                # to cover it.
                seq = [3, 2, 1, 0]

                def items_of(ch):
                    its = [("j", ch, j)
                           for j in (list(range(4 * ch + 4, SBLK))
                                     + list(range(4 * ch, 4 * ch + 4)))]
                    return its + [("norm", ch), ("c", ch)]

                work = {ch: items_of(ch) for ch in seq}
                order = []
                for i, ch in enumerate(seq):
                    items = work[ch]
                    nxt = work[seq[i + 1]] if i + 1 < len(seq) else []
                    pre = nxt[:4] if nxt else []
                    del nxt[:4]
                    order.extend(items[:-2])
                    order.extend(pre[:2])
                    order.append(items[-2])       # norm
                    order.extend(pre[2:4])
                    order.append(items[-1])       # C
                # attn lags the scores by up to 2 j-items: the
                # exp/mask chains span 2-3 engine hops, so one item of
                # PE work is not always enough to cover them
                pending = []   # (ch, j, pts) whose attn is not yet out
                for item in order:
                    if item[0] == "j":
                        ch, j = item[1], item[2]
                        pts = emit_scores(ch, j)
                        pending.append((ch, j, pts))
                        while len(pending) > 2:
                            emit_attn(*pending.pop(0))
                    elif item[0] == "norm":
                        ch = item[1]
                        mine = [r for r in pending if r[0] == ch]
                        pending = [r for r in pending if r[0] != ch]
                        for r in mine:
                            emit_attn(*r)
                        emit_norm(ch, last=(ch == seq[-1]))
                    else:
                        emit_c(item[1])
                for r in pending:
                    emit_attn(*r)

    nc.finalize()
    return nc


def _prep_in_maps(inputs, Wq, bq, Wk, bk, Wv, bv, Wo, bo):
    import ml_dtypes
    bf16 = ml_dtypes.bfloat16

    in_maps = []
    # [D, S] -> [n, c, p, f] contiguous n-groups (see kernel docstring)
    xTs = [np.ascontiguousarray(
        inputs[b].T.astype(bf16).reshape(CHUNKS, 128, NCH, 512)
        .transpose(2, 0, 1, 3)) for b in range(B)]

    def _wswiz(w):
        # [D, M] -> [128, CHUNKS*M]: SBUF tile layout, fully contiguous
        m = w.shape[1]
        return np.ascontiguousarray(
            w.astype(bf16).reshape(CHUNKS, 128, m)
            .transpose(1, 0, 2).reshape(128, CHUNKS * m))

    for core in range(NCORES):
        b = core // (NCORES // B)
        g = core % (NCORES // B)
        cols = slice(g * HPC * DH, (g + 1) * HPC * DH)
        bq_c = bq[cols].reshape(NPAIR, 128).T          # [128, 2]
        bk_c = bk[cols].reshape(NPAIR, 128).T
        bqk_c = np.ascontiguousarray(
            np.concatenate([bq_c, bk_c], axis=1), dtype=np.float32)
        bvb_c = np.ascontiguousarray(
            np.broadcast_to(bv[cols][None, :], (128, HPC * DH))).astype(bf16)
        wo_c = np.ascontiguousarray(
            Wo[cols, :].astype(bf16).reshape(NPAIR, 128, D)
            .transpose(1, 0, 2).reshape(128, NPAIR * D))
        in_maps.append({
            "xs": xTs[b],
            "wq": _wswiz(np.ascontiguousarray(Wq[:, cols])),
            "wk": _wswiz(np.ascontiguousarray(Wk[:, cols])),
            "wv": _wswiz(np.ascontiguousarray(Wv[:, cols])),
            "wo": wo_c,
            "bqk": bqk_c,
            "bvb": bvb_c,
        })
    return in_maps


def kernel(inputs, Wq, bq, Wk, bk, Wv, bv, Wo, bo, _want_results=False,
           **_run_kwargs):
    from concourse.bass_utils import run_bass_kernel_spmd

    inputs = np.asarray(inputs, dtype=np.float32)
    Wq, bq = np.asarray(Wq, np.float32), np.asarray(bq, np.float32)
    Wk, bk = np.asarray(Wk, np.float32), np.asarray(bk, np.float32)
    Wv, bv = np.asarray(Wv, np.float32), np.asarray(bv, np.float32)
    Wo, bo = np.asarray(Wo, np.float32), np.asarray(bo, np.float32)

    if "nc" not in _CACHE:
        _CACHE["nc"] = _build_nc()
    nc = _CACHE["nc"]

    in_maps = _prep_in_maps(inputs, Wq, bq, Wk, bk, Wv, bv, Wo, bo)
    res = run_bass_kernel_spmd(nc, in_maps, core_ids=list(range(NCORES)),
                               **_run_kwargs)

    out = np.zeros((B, S, D), dtype=np.float32)
    for core in range(NCORES):
        b = core // (NCORES // B)
        out[b] += res.results[core]["out"].astype(np.float32)
    out += bo[None, None, :]
    # exact last row (fully masked -> uniform attention = mean(V) @ Wo)
    for b in range(B):
        v_mean = inputs[b].mean(axis=0) @ Wv + bv
        out[b, S - 1, :] = v_mean @ Wo + bo
    if _want_results:
        return out, res
    return out

